# revision 73
# baseline (speedup 1.0000x reference)
"""Trainium2 Bass kernel: GPT-2-style causal multi-head attention.

Problem: B=4, S=2048, D=1024, H=16 heads (head_dim 64), fp32 in/out.
  q/k/v = x @ W{q,k,v} + b{q,k,v}; causal softmax attention; out = attn @ Wo + bo.

Sharding (8 cores): batch x head-group. Core c owns batch b = c//2 and head
group g = c%2 (8 heads, 512 feature dims). Wq/Wk/Wv column-sliced, Wo
row-sliced per core. Each core emits a partial o_proj output out_t [D, S]
(transposed); the host sums the two partials of each batch, transposes, and
adds bo.

Precision: q/k projections run in fp8e4m3 with DoubleRow (256-deep
contraction per matmul, 2x PE throughput; weights pre-scaled x64 on the
host, descaled inside the exp); everything else runs in bf16 with fp32 PSUM
accumulation. Measured error vs the fp32 reference: ~1.3e-2 max-rel
(gate 2e-2).

On-chip layout: transposed ([feature, seq]) so contractions sit on SBUF
partitions:
  x^T (PE transpose) -> q^T/k^T/v^T = W^T x^T -> scores^T[k, q] per head ->
  exp on ACT -> P^T -> AV in natural orientation (lhsT = P^T block, rhs = V
  block [128 k, 64]; denominators via a second matmul against a ones column
  sharing the same stationary) -> acc[q, :] with per-q denominators ->
  per-partition normalize on DVE -> attn [q, j] -> PE transpose per
  head-pair -> attn^T[j, s] -> out^T = Wo^T attn^T.

Schedule: head-pair cycles - pair jb's ACT-bound attention overlaps pair
jb+1's PE-bound projections, which are deferred as tagged thunks drained
one-or-two per kb iteration (tags + force_drain gates guarantee writers are
emitted before their consumers - the Tile framework derives dependencies
from emission order). Each head runs two q-half passes so its 8 live PSUM
accumulators occupy a single bank as one accumulation group (the HW clears
a whole 2KB zero-region on matmul start), leaving PSUM for [128,1024]
double-buffered score tiles (fewer, larger exp ops - ACT pays ~290ns fixed
per op). o_proj of q-chunks 0-1 drains through the last pair's pass-1;
chunks 2-3 form the tail.
"""

import sys

sys.path.insert(0, "/opt/trn_rl_repo")

import numpy as np
import ml_dtypes

import concourse.bass as bass
import concourse.bacc as bacc
import concourse.tile as tile
import concourse.mybir as mybir
from concourse.bass_utils import run_bass_kernel_spmd

F32 = mybir.dt.float32
B16 = mybir.dt.bfloat16
F8 = mybir.dt.float8e4
BF = ml_dtypes.bfloat16
F8NP = ml_dtypes.float8_e4m3
W8_SCALE = 64.0  # host pre-scales Wq/Wk (and bq/bk) into fp8 range
EXP_SCALE = 0.125 / (W8_SCALE * W8_SCALE)

B, S, D, H = 4, 2048, 1024, 16
HD = D // H  # 64
N_CORES = 8
HPC = 8  # heads per core
J = HPC * HD  # per-core feature dims = 512
NB = S // 128  # 16 s-blocks
NC = S // 512  # 4 chunks of 512
NJ = J // 128  # 4 j-blocks (head pairs)
ND = D // 128  # 8 d-blocks


def build_kernel():
    nc = bacc.Bacc(
        "TRN2", target_bir_lowering=False, debug=False, enable_asserts=False,
        num_devices=N_CORES,
    )

    x_d = nc.dram_tensor("x", [S, D], B16, kind="ExternalInput").ap()
    wq_d = nc.dram_tensor("wq", [D, J], F8, kind="ExternalInput").ap()
    wk_d = nc.dram_tensor("wk", [D, J], F8, kind="ExternalInput").ap()
    wv_d = nc.dram_tensor("wv", [D, J], B16, kind="ExternalInput").ap()
    wo_d = nc.dram_tensor("wo", [J, D], B16, kind="ExternalInput").ap()
    bq_d = nc.dram_tensor("bq", [J], F32, kind="ExternalInput").ap()
    bk_d = nc.dram_tensor("bk", [J], F32, kind="ExternalInput").ap()
    bv_d = nc.dram_tensor("bv", [J], F32, kind="ExternalInput").ap()
    out_d = nc.dram_tensor("out_t", [D, S], F32, kind="ExternalOutput").ap()

    with tile.TileContext(nc) as tc:
        _emit(tc, nc, x_d, wq_d, wk_d, wv_d, wo_d, bq_d, bk_d, bv_d, out_d)

    nc.compile()
    return nc


def _emit(tc, nc, x_d, wq_d, wk_d, wv_d, wo_d, bq_d, bk_d, bv_d, out_d):
    from contextlib import ExitStack

    ctx = ExitStack()
    with ctx:
        const = ctx.enter_context(tc.tile_pool(name="const", bufs=1))
        wpool = ctx.enter_context(tc.tile_pool(name="w", bufs=1))
        xspool = ctx.enter_context(tc.tile_pool(name="xs", bufs=8))
        xtpool = ctx.enter_context(tc.tile_pool(name="xt", bufs=32))
        xt8pool = ctx.enter_context(tc.tile_pool(name="xt8", bufs=16))
        qkvpool = ctx.enter_context(tc.tile_pool(name="qkv", bufs=12))
        vepool = ctx.enter_context(tc.tile_pool(name="ve", bufs=4))
        ptpool = ctx.enter_context(tc.tile_pool(name="pt", bufs=6))
        attpool = ctx.enter_context(tc.tile_pool(name="att", bufs=4))
        atpool = ctx.enter_context(tc.tile_pool(name="at", bufs=4))
        recpool = ctx.enter_context(tc.tile_pool(name="rec", bufs=8))
        stgpool = ctx.enter_context(tc.tile_pool(name="stg", bufs=4))
        ps_mm = ctx.enter_context(tc.tile_pool(name="ps_mm", bufs=2, space="PSUM"))
        ps_st = ctx.enter_context(tc.tile_pool(name="ps_st", bufs=2, space="PSUM"))
        ps_acc = ctx.enter_context(tc.tile_pool(name="ps_acc", bufs=1, space="PSUM"))

        # --- constants ---------------------------------------------------
        # identity[p, f] = 1 if p == f else 0   (for PE transpose)
        ident_f = const.tile([128, 128], F32, tag="ident_f")
        nc.gpsimd.memset(ident_f[:], 1.0)
        nc.gpsimd.affine_select(
            ident_f[:], ident_f[:], pattern=[[1, 128]],
            compare_op=mybir.AluOpType.is_equal, fill=0.0,
            base=0, channel_multiplier=-1,
        )
        ident = const.tile([128, 128], B16, tag="ident")
        nc.vector.tensor_copy(ident[:], ident_f[:])
        # causal mask for diagonal 128x128 blocks of S^T[k, q]:
        # keep (1.0) where k <= q i.e. f - p >= 0
        mask_f = const.tile([128, 128], F32, tag="mask_f")
        nc.gpsimd.memset(mask_f[:], 1.0)
        nc.gpsimd.affine_select(
            mask_f[:], mask_f[:], pattern=[[1, 128]],
            compare_op=mybir.AluOpType.is_ge, fill=0.0,
            base=0, channel_multiplier=-1,
        )
        mask = const.tile([128, 128], B16, tag="mask")
        nc.vector.tensor_copy(mask[:], mask_f[:])
        # ones column for the softmax-denominator matmul
        ones_f = const.tile([128, 1], F32, tag="ones_f")
        nc.gpsimd.memset(ones_f[:], 1.0)
        ones1 = const.tile([128, 1], B16, tag="ones1")
        nc.vector.tensor_copy(ones1[:], ones_f[:])

        # --- input + weight loads ----------------------------------------
        # x rows for the first chunk go first so the PE can start its
        # transposes while the (9x larger) weight DMAs stream in behind.
        xs_chunks = [[] for _ in range(NC)]
        for c in (0, 1):
            for si in range(4):
                t = xspool.tile([128, D], B16, tag="xs", name="xs")
                r0 = (c * 4 + si) * 128
                nc.sync.dma_start(t[:], x_d[r0: r0 + 128, :])
                xs_chunks[c].append(t)

        # wq/wk/wv: [D, J] bf16; lhsT tile [128, ND*J]: d-block ib at cols
        # [J*ib, J*ib+J).
        w_tiles = {}
        for name, wd, wdt in (
                ("q", wq_d, F8), ("k", wk_d, F8), ("v", wv_d, B16)):
            t = wpool.tile([128, ND * J], wdt, tag=f"w{name}", name=f"w{name}")
            w_tiles[name] = t

        def load_w_slice(jb):
            # per-jb column slices so head-pair 0's weights land first
            for name, wd in (("q", wq_d), ("k", wk_d), ("v", wv_d)):
                nc.sync.dma_start(
                    w_tiles[name][:].rearrange(
                        "p (ib jb j) -> p ib jb j", jb=NJ, j=128)[:, :, jb, :],
                    wd.rearrange(
                        "(ib p) (jb j) -> p ib jb j", p=128, j=128)[:, :, jb, :],
                )

        load_w_slice(0)

        # biases as [128, NJ] (col jb = partitions of j-block jb)
        bias = {}
        for name, bd in (("q", bq_d), ("k", bk_d), ("v", bv_d)):
            t = const.tile([128, NJ], F32, tag=f"b{name}")
            nc.sync.dma_start(t[:], bd.rearrange("(a p) -> p a", p=128))
            bias[name] = t

        # --- phase A: x^T for all chunks (xt tiles live for all jb) ------
        proj = {
            name: [
                qkvpool.tile([128, S], B16, tag="qkv", name=f"{name}t{jb}")
                for jb in range(NJ)
            ]
            for name in ("q", "k", "v")
        }
        qt, kt, vt = proj["q"], proj["k"], proj["v"]
        # ve[jb]: [128 k, S]; k-block sb at cols [128*sb, ...), head-pair
        # values side by side (head 2jb at +0..64, head 2jb+1 at +64..128).
        ve = [
            vepool.tile([128, S], B16, tag="ve", name=f"ve{jb}")
            for jb in range(NJ)
        ]
        xt_all = [[None] * ND for _ in range(NC)]  # [c][ib] bf16 [128, 512]
        xt8_all = [[None] * (ND // 2) for _ in range(NC)]  # [c][i2] fp8 pairs
        xs_tiles = {}

        def emit_xs_dma(c):
            xs = []
            for si in range(4):
                t = xspool.tile([128, D], B16, tag="xs", name="xs")
                r0 = (c * 4 + si) * 128
                nc.sync.dma_start(t[:], x_d[r0: r0 + 128, :])
                xs.append(t)
            xs_tiles[c] = xs

        def emit_A_pair(c, i2, use_act):
            # transpose x rows -> x^T for d-blocks 2*i2, 2*i2+1 of chunk c.
            # Chunks 0-1 run before attention: borrow the idle score-tile
            # banks so the transpose->copy pipeline is 4 deep and ps_mm
            # stays free for the first projection groups.
            t8 = xt8pool.tile([128, 1024], F8, tag="xt8", name="xt8")
            xt8_all[c][i2] = t8
            for ib in (2 * i2, 2 * i2 + 1):
                if c < 2:
                    pst = ps_st.tile([128, 512], B16, tag="ps_st", name="psta")
                else:
                    pst = ps_mm.tile([128, 512], B16, tag="ps_mm", name="pst")
                for si in range(4):
                    nc.tensor.transpose(
                        pst[:, si * 128:(si + 1) * 128],
                        xs_tiles[c][si][:, ib * 128:(ib + 1) * 128],
                        ident[:],
                    )
                t = xtpool.tile([128, 512], B16, tag="xt", name="xt")
                xt_all[c][ib] = t
                nc.vector.tensor_copy(
                    t8[:, (ib % 2) * 512:(ib % 2) * 512 + 512], pst[:])
                if use_act:
                    # ACT is idle this early - offload the bf16 copies
                    nc.scalar.copy(t[:], pst[:])
                else:
                    nc.vector.tensor_copy(t[:], pst[:])

        xs_tiles[0] = xs_chunks[0]
        xs_tiles[1] = xs_chunks[1]
        for c in (0, 1):
            for i2 in range(ND // 2):
                emit_A_pair(c, i2, use_act=True)

        # wo: [J, D] bf16; lhsT tile [128, NJ*D]: j-block jb at cols
        # [D*jb, ...). DMA'd late (cycle 2) - it is only needed by o_proj.
        wo_t = wpool.tile([128, NJ * D], B16, tag="wo")

        def load_wo():
            nc.sync.dma_start(
                wo_t[:].rearrange("p (jb o) -> p jb o", o=D),
                wo_d.rearrange("(jb p) o -> p jb o", p=128),
            )

        att_pair = [
            attpool.tile([128, S], B16, tag="att", name=f"att{jb}")
            for jb in range(NJ)
        ]
        attnT = [
            atpool.tile([128, S], B16, tag="at", name=f"at{jb}")
            for jb in range(NJ)
        ]

        def emit_oproj_group(c, ob):
            po = ps_mm.tile([128, 512], F32, tag="ps_mm", name="po")
            for jb2 in range(NJ):
                nc.tensor.matmul(
                    po[:],
                    wo_t[:, jb2 * D + ob * 128: jb2 * D + (ob + 1) * 128],
                    attnT[jb2][:, c * 512:(c + 1) * 512],
                    start=(jb2 == 0), stop=(jb2 == NJ - 1),
                )
            sg = stgpool.tile([128, 512], F32, tag="stg")
            nc.vector.tensor_copy(sg[:], po[:])
            nc.sync.dma_start(
                out_d[ob * 128:(ob + 1) * 128, c * 512:(c + 1) * 512], sg[:])

        def emit_att_transpose(jb, qb):
            tp = ps_mm.tile([128, 128], B16, tag="ps_mm", name="tpa")
            nc.tensor.transpose(
                tp[:], att_pair[jb][:, qb * 128:(qb + 1) * 128], ident[:])
            nc.vector.tensor_copy(
                attnT[jb][:, qb * 128:(qb + 1) * 128], tp[:])

        # Deferred PE work (projections, transposes, o_proj groups),
        # drained into later kb-loop iterations where ACT's exp stream is
        # the local bottleneck and the PE has slack. Entries are tagged;
        # force_drain(tags) emits everything a pass depends on BEFORE its
        # consumers are emitted (the tile framework tracks dependencies by
        # emission order - a read emitted before its writer is a race).
        pending = []
        # the previous pass's final AV group + normalize, deferred into the
        # next pass's first kb iteration (emitted after its scores/exp) so
        # the PE never blocks ACT at a pass boundary
        deferred_tail = []

        def flush_deferred():
            while deferred_tail:
                deferred_tail.pop(0)()

        def _prio(tag):
            # lower = drain sooner: pair jb's chunk-0/1 projections gate its
            # pass 0 (key 2jb), its chunk-2/3 projections gate its pass 1
            # (key 2jb+1); the chunk 2-3 x^T precedes all c23 projections.
            if isinstance(tag, tuple):
                if tag[0] == "c01":
                    return (2 * tag[1], 0)
                if tag[0] == "c23":
                    return (2 * tag[1] + 1, 1)
                if tag[0] == "tpa":
                    return (8, tag[1])
            if tag == "A":
                return (0, 1)
            return (9, 0)

        def drain(n=1):
            for _ in range(min(n, len(pending))):
                idx = min(range(len(pending)),
                          key=lambda i: (_prio(pending[i][0]), i))
                _, f, args = pending.pop(idx)
                f(*args)

        def force_drain(tags):
            rest = []
            for tag, f, args in pending:
                if tag in tags:
                    f(*args)
                else:
                    rest.append((tag, f, args))
            pending[:] = rest

        # --- jb-pipelined cycles: projections(jb) then attention of its
        # --- two heads; pair jb+1's PE-heavy projections overlap pair jb's
        # --- ACT-heavy attention.
        def emit_proj_group(jb, c, name):
            pacc = ps_mm.tile([128, 512], F32, tag="ps_mm", name="pacc")
            if name in ("q", "k"):
                # fp8 DoubleRow: 256-deep contraction per matmul
                w3 = w_tiles[name][:].rearrange(
                    "p (ib jb j) -> p ib jb j", jb=NJ, j=128)
                for i2 in range(ND // 2):
                    nc.tensor.matmul(
                        pacc[:],
                        w3[:, 2 * i2:2 * i2 + 2, jb, :],
                        xt8_all[c][i2][:].rearrange("p (k s) -> p k s", k=2),
                        start=(i2 == 0), stop=(i2 == ND // 2 - 1),
                        perf_mode=mybir.MatmulPerfMode.DoubleRow,
                    )
            else:
                for ib in range(ND):
                    nc.tensor.matmul(
                        pacc[:],
                        w_tiles[name][:, ib * J + jb * 128: ib * J + (jb + 1) * 128],
                        xt_all[c][ib][:],
                        start=(ib == 0), stop=(ib == ND - 1),
                    )
            nc.vector.tensor_scalar_add(
                proj[name][jb][:, c * 512:(c + 1) * 512], pacc[:],
                bias[name][:, jb:jb + 1],
            )

        def emit_ve_transpose(jb, sb):
            tp = ps_mm.tile([128, 128], B16, tag="ps_mm", name="tpv")
            nc.tensor.transpose(
                tp[:], vt[jb][:, sb * 128:(sb + 1) * 128], ident[:])
            nc.vector.tensor_copy(
                ve[jb][:, sb * 128:(sb + 1) * 128], tp[:])

        def queue_projections(jb):
            for c in range(NC):
                for name in ("q", "k", "v"):
                    pending.append((emit_proj_group, (jb, c, name)))
                    if name == "v":
                        for sb in range(c * 4, c * 4 + 4):
                            pending.append((emit_ve_transpose, (jb, sb)))

        def emit_attention(jb, h, p):
            par = h % 2
            hp = slice(par * 64, par * 64 + 64)
            kt_h, qt_h = kt[jb], qt[jb]
            q0 = 1024 * p
            accs = []

            def emit_av(kb, pt):
                if kb == "merged":
                    # last two k-blocks of the pass share one packed pt tile:
                    # kb=8p+6 at cols [0:256] (q-local 768..1024), kb=8p+7 at
                    # [256:384] (q-local 896..1024)
                    for kbx, base, lox in (
                            (8 * p + 6, 0, 768), (8 * p + 7, 256, 896)):
                        for qb in range(max(kbx, 8 * p), 8 * p + 8):
                            off = base + qb * 128 - q0 - lox
                            q8 = qb - 8 * p
                            grp_stop = (kbx == 8 * p + 7 and qb == 8 * p + 7)
                            acc_v, acc_d = accs
                            nc.tensor.matmul(
                                acc_v[:, q8 * 64:(q8 + 1) * 64],
                                pt[:, off:off + 128],
                                ve[jb][:, kbx * 128 + par * 64:
                                       kbx * 128 + par * 64 + 64],
                                start=False, stop=grp_stop,
                                skip_group_check=True,
                            )
                            nc.tensor.matmul(
                                acc_d[:, q8:q8 + 1],
                                pt[:, off:off + 128],
                                ones1[:],
                                start=False, stop=grp_stop,
                                skip_group_check=True,
                            )
                    return
                if not accs:
                    # allocated lazily: the first AV is emitted after the
                    # previous pass's deferred normalize, so the slot-reuse
                    # wait sees the full accessor history
                    accs.append(
                        ps_acc.tile([128, 512], F32, tag="acc_v", name="acc_v"))
                    accs.append(
                        ps_acc.tile([128, 8], F32, tag="acc_d", name="acc_d"))
                acc_v, acc_d = accs
                for qb in range(max(kb, 8 * p), 8 * p + 8):
                    off = qb * 128 - q0
                    q8 = qb - 8 * p
                    # One PSUM accumulation group per bank (HW clears the
                    # whole 2KB zero-region on start): start only on the
                    # first matmul into the bank, stop on the last. First
                    # write to each element overwrites (has_written clear),
                    # later writes accumulate.
                    grp_start = kb == 0 and qb == 8 * p
                    grp_stop = kb == 8 * p + 7 and qb == 8 * p + 7
                    nc.tensor.matmul(
                        acc_v[:, q8 * 64:(q8 + 1) * 64],
                        pt[:, off:off + 128],
                        ve[jb][:, kb * 128 + par * 64: kb * 128 + par * 64 + 64],
                        start=grp_start, stop=grp_stop,
                        skip_group_check=True,
                    )
                    nc.tensor.matmul(
                        acc_d[:, q8:q8 + 1],
                        pt[:, off:off + 128],
                        ones1[:],
                        start=grp_start, stop=grp_stop,
                        skip_group_check=True,
                    )

            # kb loop software-pipelined by one stage: AV matmuls of kb-1
            # are emitted after scores/exp of kb, so the PE's in-order
            # stream never blocks ACT behind the AV group's WAR wait on the
            # previous pass's normalize.
            av_prev = None
            for kb in range(8 + 8 * p):
                if kb == 8 * p + 6:
                    # pack the two smallest score segments (256 + 128 cols)
                    # into one tile: one exp instead of two (ACT pays ~290ns
                    # fixed per op)
                    st = ps_st.tile([128, 1024], F32, tag="ps_st")
                    for kbx, base, lox in (
                            (kb, 0, 768), (kb + 1, 256, 896)):
                        nc.tensor.matmul(
                            st[:, base:base + 1024 - lox],
                            kt_h[hp, kbx * 128:(kbx + 1) * 128],
                            qt_h[hp, q0 + lox: q0 + 1024],
                            start=True, stop=True,
                        )
                    pt = ptpool.tile([128, 1024], B16, tag="pt")
                    nc.scalar.activation(
                        pt[:, 0:384], st[:, 0:384],
                        mybir.ActivationFunctionType.Exp,
                        scale=EXP_SCALE,
                    )
                    nc.vector.tensor_mul(
                        pt[:, 0:128], pt[:, 0:128], mask[:])
                    nc.vector.tensor_mul(
                        pt[:, 256:384], pt[:, 256:384], mask[:])
                    if av_prev is not None:
                        emit_av(*av_prev)
                    drain((2 if len(pending) > 16 else 1) if p == 1 else 1)
                    av_prev = ("merged", pt)
                    break
                lo = max(kb * 128, q0)
                l0 = lo - q0  # col offset within the q-half
                st = ps_st.tile([128, 1024], F32, tag="ps_st")
                for half in range(2):
                    hlo = max(l0, 512 * half)
                    hhi = 512 * (half + 1)
                    if hlo >= hhi:
                        continue
                    nc.tensor.matmul(
                        st[:, hlo:hhi],
                        kt_h[hp, kb * 128:(kb + 1) * 128],
                        qt_h[hp, q0 + hlo: q0 + hhi],
                        start=True, stop=True,
                    )
                pt = ptpool.tile([128, 1024], B16, tag="pt")
                nc.scalar.activation(
                    pt[:, l0:1024], st[:, l0:1024],
                    mybir.ActivationFunctionType.Exp,
                    scale=EXP_SCALE,
                )
                if kb * 128 >= q0:
                    # diagonal block: mask the lower triangle
                    nc.vector.tensor_mul(
                        pt[:, l0:l0 + 128], pt[:, l0:l0 + 128], mask[:])
                if kb == 0:
                    flush_deferred()
                if av_prev is not None:
                    emit_av(*av_prev)
                drain((2 if len(pending) > 16 else 1) if p == 1 else 1)
                av_prev = (kb, pt)

            def tail():
                emit_av(*av_prev)
                acc_v, acc_d = accs
                # normalize the 8 q-blocks of this pass
                rec = recpool.tile([128, 8], F32, tag="rec")
                nc.vector.reciprocal(rec[:], acc_d[:])
                for q8 in range(8):
                    qb = 8 * p + q8
                    nc.vector.tensor_scalar_mul(
                        att_pair[jb][:, qb * 128 + par * 64: qb * 128 + par * 64 + 64],
                        acc_v[:, q8 * 64:(q8 + 1) * 64],
                        rec[:, q8:q8 + 1],
                    )
                if par == 1:
                    # head pair complete for this pass's q-blocks
                    for q8 in range(8):
                        pending.append(
                            (("tpa", p), emit_att_transpose, (jb, 8 * p + q8)))

            deferred_tail.append(tail)

        # ---- jb-pipelined cycles: each pair's projections drain through
        # ---- the previous pair's ACT-bound attention; within a cycle the
        # ---- two heads run pass 0 then pass 1 (p-grouped).
        for jb in range(NJ):
            if jb + 1 < NJ:
                load_w_slice(jb + 1)
            if jb == 0:
                for c in (0, 1):
                    for name in ("q", "k", "v"):
                        emit_proj_group(0, c, name)
                        if name == "v":
                            for sb in range(c * 4, c * 4 + 4):
                                emit_ve_transpose(0, sb)
                emit_xs_dma(2)
                emit_xs_dma(3)
                for c in (2, 3):
                    for i2 in range(ND // 2):
                        pending.append(("A", emit_A_pair, (c, i2, True)))
                for c in (2, 3):
                    pending.append(
                        (("c23", 0), emit_proj_group, (0, c, "q")))
                    pending.append(
                        (("c23", 0), emit_proj_group, (0, c, "k")))
                for c in (2, 3):
                    pending.append(
                        (("c23", 0), emit_proj_group, (0, c, "v")))
                    for sb in range(c * 4, c * 4 + 4):
                        pending.append(
                            (("c23", 0), emit_ve_transpose, (0, sb)))
            if jb == 2:
                load_wo()
            if jb + 1 < NJ:
                for c in (0, 1):
                    for name in ("q", "k", "v"):
                        pending.append(
                            (("c01", jb + 1), emit_proj_group,
                             (jb + 1, c, name)))
                        if name == "v":
                            for sb in range(c * 4, c * 4 + 4):
                                pending.append(
                                    (("c01", jb + 1), emit_ve_transpose,
                                     (jb + 1, sb)))
                for c in (2, 3):
                    pending.append(
                        (("c23", jb + 1), emit_proj_group, (jb + 1, c, "q")))
                    pending.append(
                        (("c23", jb + 1), emit_proj_group, (jb + 1, c, "k")))
                for c in (2, 3):
                    pending.append(
                        (("c23", jb + 1), emit_proj_group, (jb + 1, c, "v")))
                    for sb in range(c * 4, c * 4 + 4):
                        pending.append(
                            (("c23", jb + 1), emit_ve_transpose,
                             (jb + 1, sb)))
            force_drain({("c01", jb)})
            for h in (2 * jb, 2 * jb + 1):
                emit_attention(jb, h, 0)
            if jb == NJ - 1:
                flush_deferred()
                force_drain({("tpa", 0)})
                for cc in (0, 1):
                    for ob in range(ND):
                        pending.append(("F", emit_oproj_group, (cc, ob)))
            force_drain({"A", ("c23", jb)})
            for h in (2 * jb, 2 * jb + 1):
                emit_attention(jb, h, 1)
        flush_deferred()
        force_drain({("tpa", 1)})
        for cc in (2, 3):
            for ob in range(ND):
                pending.append(("F", emit_oproj_group, (cc, ob)))
        drain(len(pending))


_NC_CACHE = None


def _get_nc():
    global _NC_CACHE
    if _NC_CACHE is None:
        _NC_CACHE = build_kernel()
    return _NC_CACHE


def make_in_maps(inputs):
    x = np.asarray(inputs["hidden_states"], np.float32).reshape(B, S, D)
    x16 = x.astype(BF)
    ws = {
        k: np.asarray(inputs[k], np.float32).astype(BF)
        for k in ("Wv", "Wo")
    }
    for k in ("Wq", "Wk"):
        ws[k] = (np.asarray(inputs[k], np.float32) * W8_SCALE).astype(F8NP)
    bs = {k: np.asarray(inputs[k], np.float32) for k in ("bq", "bk", "bv")}
    bs["bq"] = bs["bq"] * W8_SCALE
    bs["bk"] = bs["bk"] * W8_SCALE
    in_maps = []
    for c in range(N_CORES):
        b, g = c // 2, c % 2
        js = slice(g * J, (g + 1) * J)
        in_maps.append({
            "x": np.ascontiguousarray(x16[b]),
            "wq": np.ascontiguousarray(ws["Wq"][:, js]),
            "wk": np.ascontiguousarray(ws["Wk"][:, js]),
            "wv": np.ascontiguousarray(ws["Wv"][:, js]),
            "wo": np.ascontiguousarray(ws["Wo"][js, :]),
            "bq": np.ascontiguousarray(bs["bq"][js]),
            "bk": np.ascontiguousarray(bs["bk"][js]),
            "bv": np.ascontiguousarray(bs["bv"][js]),
        })
    return in_maps


def gather_output(results, bo):
    out = np.empty((B, S, D), np.float32)
    for b in range(B):
        o = results[2 * b]["out_t"].astype(np.float32) + \
            results[2 * b + 1]["out_t"].astype(np.float32)
        out[b] = o.T + bo[None, :]
    return out


def kernel(**inputs) -> np.ndarray:
    nc = _get_nc()
    in_maps = make_in_maps(inputs)
    res = run_bass_kernel_spmd(nc, in_maps, core_ids=list(range(N_CORES)))
    bo = np.asarray(inputs["bo"], np.float32)
    return gather_output(res.results, bo)


if __name__ == "__main__":
    rng = np.random.default_rng(0)
    ins = {
        "hidden_states": rng.standard_normal((B, S, D)).astype(np.float32),
        "Wq": (rng.standard_normal((D, D)) * 0.02).astype(np.float32),
        "bq": np.zeros(D, np.float32),
        "Wk": (rng.standard_normal((D, D)) * 0.02).astype(np.float32),
        "bk": np.zeros(D, np.float32),
        "Wv": (rng.standard_normal((D, D)) * 0.02).astype(np.float32),
        "bv": np.zeros(D, np.float32),
        "Wo": (rng.standard_normal((D, D)) * 0.02).astype(np.float32),
        "bo": np.zeros(D, np.float32),
    }
    out = kernel(**ins)
    print("out", out.shape, out.dtype, float(np.abs(out).mean()))


# revision 74
# speedup vs baseline: 1.0137x; 1.0137x over previous
"""Trainium2 Bass kernel: GPT-2-style causal multi-head attention.

Problem: B=4, S=2048, D=1024, H=16 heads (head_dim 64), fp32 in/out.
  q/k/v = x @ W{q,k,v} + b{q,k,v}; causal softmax attention; out = attn @ Wo + bo.

Sharding (8 cores): batch x head-group. Core c owns batch b = c//2 and head
group g = c%2 (8 heads, 512 feature dims). Wq/Wk/Wv column-sliced, Wo
row-sliced per core. Each core emits a partial o_proj output out_t [D, S]
(transposed); the host sums the two partials of each batch, transposes, and
adds bo.

Precision: q/k projections run in fp8e4m3 with DoubleRow (256-deep
contraction per matmul, 2x PE throughput; weights pre-scaled x64 on the
host, descaled inside the exp); everything else runs in bf16 with fp32 PSUM
accumulation. Measured error vs the fp32 reference: ~1.3e-2 max-rel
(gate 2e-2).

On-chip layout: transposed ([feature, seq]) so contractions sit on SBUF
partitions:
  x^T (PE transpose) -> q^T/k^T/v^T = W^T x^T -> scores^T[k, q] per head ->
  exp on ACT -> P^T -> AV in natural orientation (lhsT = P^T block, rhs = V
  block [128 k, 64]; denominators via a second matmul against a ones column
  sharing the same stationary) -> acc[q, :] with per-q denominators ->
  per-partition normalize on DVE -> attn [q, j] -> PE transpose per
  head-pair -> attn^T[j, s] -> out^T = Wo^T attn^T.

Schedule: head-pair cycles - pair jb's ACT-bound attention overlaps pair
jb+1's PE-bound projections, which are deferred as tagged thunks drained
one-or-two per kb iteration (tags + force_drain gates guarantee writers are
emitted before their consumers - the Tile framework derives dependencies
from emission order). Each head runs two q-half passes so its 8 live PSUM
accumulators occupy a single bank as one accumulation group (the HW clears
a whole 2KB zero-region on matmul start), leaving PSUM for [128,1024]
double-buffered score tiles (fewer, larger exp ops - ACT pays ~290ns fixed
per op). o_proj of q-chunks 0-1 drains through the last pair's pass-1;
chunks 2-3 form the tail.
"""

import sys

sys.path.insert(0, "/opt/trn_rl_repo")

import numpy as np
import ml_dtypes

import concourse.bass as bass
import concourse.bacc as bacc
import concourse.tile as tile
import concourse.mybir as mybir
from concourse.bass_utils import run_bass_kernel_spmd

F32 = mybir.dt.float32
B16 = mybir.dt.bfloat16
F8 = mybir.dt.float8e4
BF = ml_dtypes.bfloat16
F8NP = ml_dtypes.float8_e4m3
W8_SCALE = 64.0  # host pre-scales Wq/Wk (and bq/bk) into fp8 range
EXP_SCALE = 0.125 / (W8_SCALE * W8_SCALE)

B, S, D, H = 4, 2048, 1024, 16
HD = D // H  # 64
N_CORES = 8
HPC = 8  # heads per core
J = HPC * HD  # per-core feature dims = 512
NB = S // 128  # 16 s-blocks
NC = S // 512  # 4 chunks of 512
NJ = J // 128  # 4 j-blocks (head pairs)
ND = D // 128  # 8 d-blocks


def build_kernel():
    nc = bacc.Bacc(
        "TRN2", target_bir_lowering=False, debug=False, enable_asserts=False,
        num_devices=N_CORES,
    )

    x_d = nc.dram_tensor("x", [S, D], B16, kind="ExternalInput").ap()
    wq_d = nc.dram_tensor("wq", [D, J], F8, kind="ExternalInput").ap()
    wk_d = nc.dram_tensor("wk", [D, J], F8, kind="ExternalInput").ap()
    wv_d = nc.dram_tensor("wv", [D, J], B16, kind="ExternalInput").ap()
    wo_d = nc.dram_tensor("wo", [J, D], B16, kind="ExternalInput").ap()
    bq_d = nc.dram_tensor("bq", [J], F32, kind="ExternalInput").ap()
    bk_d = nc.dram_tensor("bk", [J], F32, kind="ExternalInput").ap()
    bv_d = nc.dram_tensor("bv", [J], F32, kind="ExternalInput").ap()
    out_d = nc.dram_tensor("out_t", [D, S], F32, kind="ExternalOutput").ap()

    with tile.TileContext(nc) as tc:
        _emit(tc, nc, x_d, wq_d, wk_d, wv_d, wo_d, bq_d, bk_d, bv_d, out_d)

    nc.compile()
    return nc


def _emit(tc, nc, x_d, wq_d, wk_d, wv_d, wo_d, bq_d, bk_d, bv_d, out_d):
    from contextlib import ExitStack

    ctx = ExitStack()
    with ctx:
        const = ctx.enter_context(tc.tile_pool(name="const", bufs=1))
        wpool = ctx.enter_context(tc.tile_pool(name="w", bufs=1))
        xspool = ctx.enter_context(tc.tile_pool(name="xs", bufs=8))
        xtpool = ctx.enter_context(tc.tile_pool(name="xt", bufs=32))
        xt8pool = ctx.enter_context(tc.tile_pool(name="xt8", bufs=16))
        qkvpool = ctx.enter_context(tc.tile_pool(name="qkv", bufs=12))
        vepool = ctx.enter_context(tc.tile_pool(name="ve", bufs=4))
        ptpool = ctx.enter_context(tc.tile_pool(name="pt", bufs=6))
        attpool = ctx.enter_context(tc.tile_pool(name="att", bufs=4))
        atpool = ctx.enter_context(tc.tile_pool(name="at", bufs=4))
        recpool = ctx.enter_context(tc.tile_pool(name="rec", bufs=8))
        stgpool = ctx.enter_context(tc.tile_pool(name="stg", bufs=4))
        ps_mm = ctx.enter_context(tc.tile_pool(name="ps_mm", bufs=2, space="PSUM"))
        ps_st = ctx.enter_context(tc.tile_pool(name="ps_st", bufs=2, space="PSUM"))
        ps_acc = ctx.enter_context(tc.tile_pool(name="ps_acc", bufs=1, space="PSUM"))

        # --- constants ---------------------------------------------------
        # identity[p, f] = 1 if p == f else 0   (for PE transpose)
        ident_f = const.tile([128, 128], F32, tag="ident_f")
        nc.gpsimd.memset(ident_f[:], 1.0)
        nc.gpsimd.affine_select(
            ident_f[:], ident_f[:], pattern=[[1, 128]],
            compare_op=mybir.AluOpType.is_equal, fill=0.0,
            base=0, channel_multiplier=-1,
        )
        ident = const.tile([128, 128], B16, tag="ident")
        nc.vector.tensor_copy(ident[:], ident_f[:])
        # causal mask for diagonal 128x128 blocks of S^T[k, q]:
        # keep (1.0) where k <= q i.e. f - p >= 0
        mask_f = const.tile([128, 128], F32, tag="mask_f")
        nc.gpsimd.memset(mask_f[:], 1.0)
        nc.gpsimd.affine_select(
            mask_f[:], mask_f[:], pattern=[[1, 128]],
            compare_op=mybir.AluOpType.is_ge, fill=0.0,
            base=0, channel_multiplier=-1,
        )
        mask = const.tile([128, 128], B16, tag="mask")
        nc.vector.tensor_copy(mask[:], mask_f[:])
        # ones column for the softmax-denominator matmul
        ones_f = const.tile([128, 1], F32, tag="ones_f")
        nc.gpsimd.memset(ones_f[:], 1.0)
        ones1 = const.tile([128, 1], B16, tag="ones1")
        nc.vector.tensor_copy(ones1[:], ones_f[:])

        # --- input + weight loads ----------------------------------------
        # x rows for the first chunk go first so the PE can start its
        # transposes while the (9x larger) weight DMAs stream in behind.
        xs_chunks = [[] for _ in range(NC)]
        for c in (0, 1):
            for si in range(4):
                t = xspool.tile([128, D], B16, tag="xs", name="xs")
                r0 = (c * 4 + si) * 128
                nc.sync.dma_start(t[:], x_d[r0: r0 + 128, :])
                xs_chunks[c].append(t)

        # wq/wk/wv: [D, J] bf16; lhsT tile [128, ND*J]: d-block ib at cols
        # [J*ib, J*ib+J).
        w_tiles = {}
        for name, wd, wdt in (
                ("q", wq_d, F8), ("k", wk_d, F8), ("v", wv_d, B16)):
            t = wpool.tile([128, ND * J], wdt, tag=f"w{name}", name=f"w{name}")
            w_tiles[name] = t

        def load_w_slice(jb):
            # per-jb column slices so head-pair 0's weights land first
            for name, wd in (("q", wq_d), ("k", wk_d), ("v", wv_d)):
                nc.sync.dma_start(
                    w_tiles[name][:].rearrange(
                        "p (ib jb j) -> p ib jb j", jb=NJ, j=128)[:, :, jb, :],
                    wd.rearrange(
                        "(ib p) (jb j) -> p ib jb j", p=128, j=128)[:, :, jb, :],
                )

        load_w_slice(0)

        # biases as [128, NJ] (col jb = partitions of j-block jb)
        bias = {}
        for name, bd in (("q", bq_d), ("k", bk_d), ("v", bv_d)):
            t = const.tile([128, NJ], F32, tag=f"b{name}")
            nc.sync.dma_start(t[:], bd.rearrange("(a p) -> p a", p=128))
            bias[name] = t

        # --- phase A: x^T for all chunks (xt tiles live for all jb) ------
        proj = {
            name: [
                qkvpool.tile([128, S], B16, tag="qkv", name=f"{name}t{jb}")
                for jb in range(NJ)
            ]
            for name in ("q", "k", "v")
        }
        qt, kt, vt = proj["q"], proj["k"], proj["v"]
        # ve[jb]: [128 k, S]; k-block sb at cols [128*sb, ...), head-pair
        # values side by side (head 2jb at +0..64, head 2jb+1 at +64..128).
        ve = [
            vepool.tile([128, S], B16, tag="ve", name=f"ve{jb}")
            for jb in range(NJ)
        ]
        xt_all = [[None] * ND for _ in range(NC)]  # [c][ib] bf16 [128, 512]
        xt8_all = [[None] * (ND // 2) for _ in range(NC)]  # [c][i2] fp8 pairs
        xs_tiles = {}

        def emit_xs_dma(c):
            xs = []
            for si in range(4):
                t = xspool.tile([128, D], B16, tag="xs", name="xs")
                r0 = (c * 4 + si) * 128
                nc.sync.dma_start(t[:], x_d[r0: r0 + 128, :])
                xs.append(t)
            xs_tiles[c] = xs

        def emit_A_pair(c, i2, use_act):
            # transpose x rows -> x^T for d-blocks 2*i2, 2*i2+1 of chunk c.
            # Chunks 0-1 run before attention: borrow the idle score-tile
            # banks so the transpose->copy pipeline is 4 deep and ps_mm
            # stays free for the first projection groups.
            t8 = xt8pool.tile([128, 1024], F8, tag="xt8", name="xt8")
            xt8_all[c][i2] = t8
            for ib in (2 * i2, 2 * i2 + 1):
                if c < 2:
                    pst = ps_st.tile([128, 512], B16, tag="ps_st", name="psta")
                else:
                    pst = ps_mm.tile([128, 512], B16, tag="ps_mm", name="pst")
                for si in range(4):
                    nc.tensor.transpose(
                        pst[:, si * 128:(si + 1) * 128],
                        xs_tiles[c][si][:, ib * 128:(ib + 1) * 128],
                        ident[:],
                    )
                t = xtpool.tile([128, 512], B16, tag="xt", name="xt")
                xt_all[c][ib] = t
                nc.vector.tensor_copy(
                    t8[:, (ib % 2) * 512:(ib % 2) * 512 + 512], pst[:])
                if use_act:
                    # ACT is idle this early - offload the bf16 copies
                    nc.scalar.copy(t[:], pst[:])
                else:
                    nc.vector.tensor_copy(t[:], pst[:])

        xs_tiles[0] = xs_chunks[0]
        xs_tiles[1] = xs_chunks[1]
        for c in (0, 1):
            for i2 in range(ND // 2):
                emit_A_pair(c, i2, use_act=True)

        # wo: [J, D] bf16; lhsT tile [128, NJ*D]: j-block jb at cols
        # [D*jb, ...). DMA'd late (cycle 2) - it is only needed by o_proj.
        wo_t = wpool.tile([128, NJ * D], B16, tag="wo")

        def load_wo():
            nc.sync.dma_start(
                wo_t[:].rearrange("p (jb o) -> p jb o", o=D),
                wo_d.rearrange("(jb p) o -> p jb o", p=128),
            )

        att_pair = [
            attpool.tile([128, S], B16, tag="att", name=f"att{jb}")
            for jb in range(NJ)
        ]
        attnT = [
            atpool.tile([128, S], B16, tag="at", name=f"at{jb}")
            for jb in range(NJ)
        ]

        def emit_oproj_group(c, ob):
            po = ps_mm.tile([128, 512], F32, tag="ps_mm", name="po")
            for jb2 in range(NJ):
                nc.tensor.matmul(
                    po[:],
                    wo_t[:, jb2 * D + ob * 128: jb2 * D + (ob + 1) * 128],
                    attnT[jb2][:, c * 512:(c + 1) * 512],
                    start=(jb2 == 0), stop=(jb2 == NJ - 1),
                )
            sg = stgpool.tile([128, 512], F32, tag="stg")
            nc.vector.tensor_copy(sg[:], po[:])
            nc.sync.dma_start(
                out_d[ob * 128:(ob + 1) * 128, c * 512:(c + 1) * 512], sg[:])

        def emit_att_transpose(jb, qb):
            tp = ps_mm.tile([128, 128], B16, tag="ps_mm", name="tpa")
            nc.tensor.transpose(
                tp[:], att_pair[jb][:, qb * 128:(qb + 1) * 128], ident[:])
            nc.vector.tensor_copy(
                attnT[jb][:, qb * 128:(qb + 1) * 128], tp[:])

        # Deferred PE work (projections, transposes, o_proj groups),
        # drained into later kb-loop iterations where ACT's exp stream is
        # the local bottleneck and the PE has slack. Entries are tagged;
        # force_drain(tags) emits everything a pass depends on BEFORE its
        # consumers are emitted (the tile framework tracks dependencies by
        # emission order - a read emitted before its writer is a race).
        pending = []
        # the previous pass's final AV group + normalize, deferred into the
        # next pass's first kb iteration (emitted after its scores/exp) so
        # the PE never blocks ACT at a pass boundary
        deferred_tail = []

        def flush_deferred():
            while deferred_tail:
                deferred_tail.pop(0)()

        def _prio(tag):
            # lower = drain sooner: pair jb's chunk-0/1 projections gate its
            # pass 0 (key 2jb), its chunk-2/3 projections gate its pass 1
            # (key 2jb+1); the chunk 2-3 x^T precedes all c23 projections.
            if isinstance(tag, tuple):
                if tag[0] == "c01":
                    return (2 * tag[1], 0)
                if tag[0] == "c23":
                    return (2 * tag[1] + 1, 1)
                if tag[0] == "tpa":
                    return (8, tag[1])
            if tag == "A":
                return (0, 1)
            return (9, 0)

        def drain(n=1):
            for _ in range(min(n, len(pending))):
                idx = min(range(len(pending)),
                          key=lambda i: (_prio(pending[i][0]), i))
                _, f, args = pending.pop(idx)
                f(*args)

        def force_drain(tags):
            rest = []
            for tag, f, args in pending:
                if tag in tags:
                    f(*args)
                else:
                    rest.append((tag, f, args))
            pending[:] = rest

        # --- jb-pipelined cycles: projections(jb) then attention of its
        # --- two heads; pair jb+1's PE-heavy projections overlap pair jb's
        # --- ACT-heavy attention.
        def emit_proj_group(jb, c, name):
            pacc = ps_mm.tile([128, 512], F32, tag="ps_mm", name="pacc")
            if name in ("q", "k"):
                # fp8 DoubleRow: 256-deep contraction per matmul
                w3 = w_tiles[name][:].rearrange(
                    "p (ib jb j) -> p ib jb j", jb=NJ, j=128)
                for i2 in range(ND // 2):
                    nc.tensor.matmul(
                        pacc[:],
                        w3[:, 2 * i2:2 * i2 + 2, jb, :],
                        xt8_all[c][i2][:].rearrange("p (k s) -> p k s", k=2),
                        start=(i2 == 0), stop=(i2 == ND // 2 - 1),
                        perf_mode=mybir.MatmulPerfMode.DoubleRow,
                    )
            else:
                for ib in range(ND):
                    nc.tensor.matmul(
                        pacc[:],
                        w_tiles[name][:, ib * J + jb * 128: ib * J + (jb + 1) * 128],
                        xt_all[c][ib][:],
                        start=(ib == 0), stop=(ib == ND - 1),
                    )
            nc.vector.tensor_scalar_add(
                proj[name][jb][:, c * 512:(c + 1) * 512], pacc[:],
                bias[name][:, jb:jb + 1],
            )

        def emit_ve_transpose(jb, sb):
            tp = ps_mm.tile([128, 128], B16, tag="ps_mm", name="tpv")
            nc.tensor.transpose(
                tp[:], vt[jb][:, sb * 128:(sb + 1) * 128], ident[:])
            nc.vector.tensor_copy(
                ve[jb][:, sb * 128:(sb + 1) * 128], tp[:])

        def queue_projections(jb):
            for c in range(NC):
                for name in ("q", "k", "v"):
                    pending.append((emit_proj_group, (jb, c, name)))
                    if name == "v":
                        for sb in range(c * 4, c * 4 + 4):
                            pending.append((emit_ve_transpose, (jb, sb)))

        def emit_attention(jb, h, p):
            par = h % 2
            hp = slice(par * 64, par * 64 + 64)
            kt_h, qt_h = kt[jb], qt[jb]
            q0 = 1024 * p
            accs = []

            def emit_av(kb, segs_or_pt, pt=None):
                if kb == "merged":
                    segs = segs_or_pt
                    for kbx, base, lox in segs:
                        for qb in range(max(kbx, 8 * p), 8 * p + 8):
                            off = base + qb * 128 - q0 - lox
                            q8 = qb - 8 * p
                            grp_stop = (kbx == 8 * p + 7 and qb == 8 * p + 7)
                            acc_v, acc_d = accs
                            nc.tensor.matmul(
                                acc_v[:, q8 * 64:(q8 + 1) * 64],
                                pt[:, off:off + 128],
                                ve[jb][:, kbx * 128 + par * 64:
                                       kbx * 128 + par * 64 + 64],
                                start=False, stop=grp_stop,
                                skip_group_check=True,
                            )
                            nc.tensor.matmul(
                                acc_d[:, q8:q8 + 1],
                                pt[:, off:off + 128],
                                ones1[:],
                                start=False, stop=grp_stop,
                                skip_group_check=True,
                            )
                    return
                pt = segs_or_pt
                if not accs:
                    # allocated lazily: the first AV is emitted after the
                    # previous pass's deferred normalize, so the slot-reuse
                    # wait sees the full accessor history
                    accs.append(
                        ps_acc.tile([128, 512], F32, tag="acc_v", name="acc_v"))
                    accs.append(
                        ps_acc.tile([128, 8], F32, tag="acc_d", name="acc_d"))
                acc_v, acc_d = accs
                for qb in range(max(kb, 8 * p), 8 * p + 8):
                    off = qb * 128 - q0
                    q8 = qb - 8 * p
                    # One PSUM accumulation group per bank (HW clears the
                    # whole 2KB zero-region on start): start only on the
                    # first matmul into the bank, stop on the last. First
                    # write to each element overwrites (has_written clear),
                    # later writes accumulate.
                    grp_start = kb == 0 and qb == 8 * p
                    grp_stop = kb == 8 * p + 7 and qb == 8 * p + 7
                    nc.tensor.matmul(
                        acc_v[:, q8 * 64:(q8 + 1) * 64],
                        pt[:, off:off + 128],
                        ve[jb][:, kb * 128 + par * 64: kb * 128 + par * 64 + 64],
                        start=grp_start, stop=grp_stop,
                        skip_group_check=True,
                    )
                    nc.tensor.matmul(
                        acc_d[:, q8:q8 + 1],
                        pt[:, off:off + 128],
                        ones1[:],
                        start=grp_start, stop=grp_stop,
                        skip_group_check=True,
                    )

            # kb loop software-pipelined by one stage: AV matmuls of kb-1
            # are emitted after scores/exp of kb, so the PE's in-order
            # stream never blocks ACT behind the AV group's WAR wait on the
            # previous pass's normalize.
            av_prev = None
            for kb in range(8 + 8 * p):
                if kb == 8 * p + 5:
                    # pack the three smallest score segments (384 + 256 +
                    # 128 cols) into one tile: one exp instead of three (ACT
                    # pays ~290ns fixed per op). Score matmuls split at the
                    # PSUM bank boundary (col 512).
                    segs = ((kb, 0, 640), (kb + 1, 384, 768),
                            (kb + 2, 640, 896))
                    st = ps_st.tile([128, 1024], F32, tag="ps_st")
                    for kbx, base, lox in segs:
                        hi = base + 1024 - lox
                        for s0, s1 in ((base, min(hi, 512)),
                                       (max(base, 512), hi)):
                            if s1 <= s0:
                                continue
                            nc.tensor.matmul(
                                st[:, s0:s1],
                                kt_h[hp, kbx * 128:(kbx + 1) * 128],
                                qt_h[hp, q0 + lox + (s0 - base):
                                     q0 + lox + (s1 - base)],
                                start=True, stop=True,
                            )
                    pt = ptpool.tile([128, 1024], B16, tag="pt")
                    nc.scalar.activation(
                        pt[:, 0:768], st[:, 0:768],
                        mybir.ActivationFunctionType.Exp,
                        scale=EXP_SCALE,
                    )
                    for kbx, base, lox in segs:
                        nc.vector.tensor_mul(
                            pt[:, base:base + 128],
                            pt[:, base:base + 128], mask[:])
                    if av_prev is not None:
                        emit_av(*av_prev)
                    drain((2 if len(pending) > 16 else 1) if p == 1 else 1)
                    av_prev = ("merged", segs, pt)
                    break
                lo = max(kb * 128, q0)
                l0 = lo - q0  # col offset within the q-half
                st = ps_st.tile([128, 1024], F32, tag="ps_st")
                for half in range(2):
                    hlo = max(l0, 512 * half)
                    hhi = 512 * (half + 1)
                    if hlo >= hhi:
                        continue
                    nc.tensor.matmul(
                        st[:, hlo:hhi],
                        kt_h[hp, kb * 128:(kb + 1) * 128],
                        qt_h[hp, q0 + hlo: q0 + hhi],
                        start=True, stop=True,
                    )
                pt = ptpool.tile([128, 1024], B16, tag="pt")
                nc.scalar.activation(
                    pt[:, l0:1024], st[:, l0:1024],
                    mybir.ActivationFunctionType.Exp,
                    scale=EXP_SCALE,
                )
                if kb * 128 >= q0:
                    # diagonal block: mask the lower triangle
                    nc.vector.tensor_mul(
                        pt[:, l0:l0 + 128], pt[:, l0:l0 + 128], mask[:])
                if kb == 0:
                    flush_deferred()
                if av_prev is not None:
                    emit_av(*av_prev)
                drain((2 if len(pending) > 16 else 1) if p == 1 else 1)
                av_prev = (kb, pt)

            def tail():
                emit_av(*av_prev)
                acc_v, acc_d = accs
                # normalize the 8 q-blocks of this pass
                rec = recpool.tile([128, 8], F32, tag="rec")
                nc.vector.reciprocal(rec[:], acc_d[:])
                for q8 in range(8):
                    qb = 8 * p + q8
                    nc.vector.tensor_scalar_mul(
                        att_pair[jb][:, qb * 128 + par * 64: qb * 128 + par * 64 + 64],
                        acc_v[:, q8 * 64:(q8 + 1) * 64],
                        rec[:, q8:q8 + 1],
                    )
                if par == 1:
                    # head pair complete for this pass's q-blocks
                    for q8 in range(8):
                        pending.append(
                            (("tpa", p), emit_att_transpose, (jb, 8 * p + q8)))

            deferred_tail.append(tail)

        # ---- jb-pipelined cycles: each pair's projections drain through
        # ---- the previous pair's ACT-bound attention; within a cycle the
        # ---- two heads run pass 0 then pass 1 (p-grouped).
        for jb in range(NJ):
            if jb + 1 < NJ:
                load_w_slice(jb + 1)
            if jb == 0:
                for c in (0, 1):
                    for name in ("q", "k", "v"):
                        emit_proj_group(0, c, name)
                        if name == "v":
                            for sb in range(c * 4, c * 4 + 4):
                                emit_ve_transpose(0, sb)
                emit_xs_dma(2)
                emit_xs_dma(3)
                for c in (2, 3):
                    for i2 in range(ND // 2):
                        pending.append(("A", emit_A_pair, (c, i2, True)))
                for c in (2, 3):
                    pending.append(
                        (("c23", 0), emit_proj_group, (0, c, "q")))
                    pending.append(
                        (("c23", 0), emit_proj_group, (0, c, "k")))
                for c in (2, 3):
                    pending.append(
                        (("c23", 0), emit_proj_group, (0, c, "v")))
                    for sb in range(c * 4, c * 4 + 4):
                        pending.append(
                            (("c23", 0), emit_ve_transpose, (0, sb)))
            if jb == 2:
                load_wo()
            if jb + 1 < NJ:
                for c in (0, 1):
                    for name in ("q", "k", "v"):
                        pending.append(
                            (("c01", jb + 1), emit_proj_group,
                             (jb + 1, c, name)))
                        if name == "v":
                            for sb in range(c * 4, c * 4 + 4):
                                pending.append(
                                    (("c01", jb + 1), emit_ve_transpose,
                                     (jb + 1, sb)))
                for c in (2, 3):
                    pending.append(
                        (("c23", jb + 1), emit_proj_group, (jb + 1, c, "q")))
                    pending.append(
                        (("c23", jb + 1), emit_proj_group, (jb + 1, c, "k")))
                for c in (2, 3):
                    pending.append(
                        (("c23", jb + 1), emit_proj_group, (jb + 1, c, "v")))
                    for sb in range(c * 4, c * 4 + 4):
                        pending.append(
                            (("c23", jb + 1), emit_ve_transpose,
                             (jb + 1, sb)))
            force_drain({("c01", jb)})
            for h in (2 * jb, 2 * jb + 1):
                emit_attention(jb, h, 0)
            if jb == NJ - 1:
                flush_deferred()
                force_drain({("tpa", 0)})
                for cc in (0, 1):
                    for ob in range(ND):
                        pending.append(("F", emit_oproj_group, (cc, ob)))
            force_drain({"A", ("c23", jb)})
            for h in (2 * jb, 2 * jb + 1):
                emit_attention(jb, h, 1)
        flush_deferred()
        force_drain({("tpa", 1)})
        for cc in (2, 3):
            for ob in range(ND):
                pending.append(("F", emit_oproj_group, (cc, ob)))
        drain(len(pending))


_NC_CACHE = None


def _get_nc():
    global _NC_CACHE
    if _NC_CACHE is None:
        _NC_CACHE = build_kernel()
    return _NC_CACHE


def make_in_maps(inputs):
    x = np.asarray(inputs["hidden_states"], np.float32).reshape(B, S, D)
    x16 = x.astype(BF)
    ws = {
        k: np.asarray(inputs[k], np.float32).astype(BF)
        for k in ("Wv", "Wo")
    }
    for k in ("Wq", "Wk"):
        ws[k] = (np.asarray(inputs[k], np.float32) * W8_SCALE).astype(F8NP)
    bs = {k: np.asarray(inputs[k], np.float32) for k in ("bq", "bk", "bv")}
    bs["bq"] = bs["bq"] * W8_SCALE
    bs["bk"] = bs["bk"] * W8_SCALE
    in_maps = []
    for c in range(N_CORES):
        b, g = c // 2, c % 2
        js = slice(g * J, (g + 1) * J)
        in_maps.append({
            "x": np.ascontiguousarray(x16[b]),
            "wq": np.ascontiguousarray(ws["Wq"][:, js]),
            "wk": np.ascontiguousarray(ws["Wk"][:, js]),
            "wv": np.ascontiguousarray(ws["Wv"][:, js]),
            "wo": np.ascontiguousarray(ws["Wo"][js, :]),
            "bq": np.ascontiguousarray(bs["bq"][js]),
            "bk": np.ascontiguousarray(bs["bk"][js]),
            "bv": np.ascontiguousarray(bs["bv"][js]),
        })
    return in_maps


def gather_output(results, bo):
    out = np.empty((B, S, D), np.float32)
    for b in range(B):
        o = results[2 * b]["out_t"].astype(np.float32) + \
            results[2 * b + 1]["out_t"].astype(np.float32)
        out[b] = o.T + bo[None, :]
    return out


def kernel(**inputs) -> np.ndarray:
    nc = _get_nc()
    in_maps = make_in_maps(inputs)
    res = run_bass_kernel_spmd(nc, in_maps, core_ids=list(range(N_CORES)))
    bo = np.asarray(inputs["bo"], np.float32)
    return gather_output(res.results, bo)


if __name__ == "__main__":
    rng = np.random.default_rng(0)
    ins = {
        "hidden_states": rng.standard_normal((B, S, D)).astype(np.float32),
        "Wq": (rng.standard_normal((D, D)) * 0.02).astype(np.float32),
        "bq": np.zeros(D, np.float32),
        "Wk": (rng.standard_normal((D, D)) * 0.02).astype(np.float32),
        "bk": np.zeros(D, np.float32),
        "Wv": (rng.standard_normal((D, D)) * 0.02).astype(np.float32),
        "bv": np.zeros(D, np.float32),
        "Wo": (rng.standard_normal((D, D)) * 0.02).astype(np.float32),
        "bo": np.zeros(D, np.float32),
    }
    out = kernel(**ins)
    print("out", out.shape, out.dtype, float(np.abs(out).mean()))
